# revision 26
# baseline (speedup 1.0000x reference)
"""AmplitudeQuantumNet Trainium2 kernel (8-core data parallel), v2.

Per core (128 samples, 4 chunks of 32):
  conv1: K=16 im2col (2q+jy rows), M=64 (jy',c), col-packed sample pairs
         -> psum [128]=(A|B); x-pool at drain (dual-psum-AP tt.max)
         -> p1x [128=(half,jy',c), 16smp, 14xp, 16q]
  fold:  3 partition-move DMAs + 2 stt(max,max) ops write relu(pool) straight
         into conv2's input tile (in2 center block, y interleaved, 2x mode)
  conv2: in2 [96=(b,ci), 16x', 32s, 18y''] with L/R blocks built by 2 fat
         contiguous DMAs; 3 accumulating dy-matmuls, col-packed pairs
         -> x-pool at drain -> y-fold -> bias+relu ACT ops that also perform
         the checkerboard pixel split into p2full [128=(par,c), 128s, 28i]
  fc:    activation-stationary: 28 K=128 chunks, moving fcw N=256
         -> psum [128smp, 256feats]; tanh; 2 PE transposes back to
         feats [128=f, 2, 128smp]
  quantum + MLP tail: host-built 256x256 unitary, as before.
"""

import sys

sys.path.insert(0, "/opt/trn_rl_repo")

import numpy as np
import ml_dtypes

BF16 = ml_dtypes.bfloat16

N_QUBITS = 8
Q_DEPTH = 10
DIM = 256
BN_EPS = 1e-5
B = 1024
NCORES = 8
B_CORE = B // NCORES    # 128
NCH = 4
CH = B_CORE // NCH      # 32 samples per chunk
HALF = CH // 2          # 16 (A/B halves)

_CACHE = {}


# ---------------------------------------------------------------- host precompute
def _quantum_unitary(q_params):
    """256x256 complex matrix of the full circuit (H layer + 10x[RX layer + diag])."""
    bits = ((np.arange(DIM)[:, None] >> (N_QUBITS - 1 - np.arange(N_QUBITS))) & 1)
    ph = np.where(np.arange(N_QUBITS) % 2 == 0, 1j, np.exp(1j * np.pi / 4))
    diag = np.prod(np.power(ph[None, :], bits), axis=1)
    cz = np.ones(DIM)
    for i, j in [(0, 1), (2, 3), (4, 5), (6, 7), (1, 2), (3, 4), (5, 6)]:
        cz = cz * ((-1.0) ** (bits[:, i] * bits[:, j]))
    diagc = (diag * cz).astype(np.complex128)

    def app(M, U, w):
        M = M.reshape((2,) * N_QUBITS + (DIM,))
        M = np.moveaxis(M, w, 0)
        M = np.tensordot(U, M, axes=(1, 0))
        M = np.moveaxis(M, 0, w)
        return M.reshape(DIM, DIM)

    M = np.eye(DIM, dtype=np.complex128)
    H = np.array([[1.0, 1.0], [1.0, -1.0]]) / np.sqrt(2.0)
    for w in range(N_QUBITS):
        M = app(M, H, w)
    qw = np.asarray(q_params, np.float64).reshape(Q_DEPTH, N_QUBITS)
    X = np.array([[0.0, 1.0], [1.0, 0.0]])
    I2 = np.eye(2)
    for layer in range(Q_DEPTH):
        for w in range(N_QUBITS):
            t = qw[layer, w]
            U = np.cos(t / 2) * I2 - 1j * np.sin(t / 2) * X
            M = app(M, U, w)
        M = diagc[:, None] * M
    zsigns = (1 - 2 * bits).astype(np.float64)  # [256, 8]
    return M, zsigns


def _host_prep(inputs):
    f32 = np.float32
    x = np.asarray(inputs["x"], f32)  # [1024,1,28,28]

    inv1 = np.asarray(inputs["bn1_gamma"], f32) / np.sqrt(np.asarray(inputs["bn1_var"], f32) + BN_EPS)
    w1f = np.asarray(inputs["conv1_w"], f32) * inv1[:, None, None, None]
    b1f = (np.asarray(inputs["conv1_b"], f32) - np.asarray(inputs["bn1_mean"], f32)) * inv1 \
        + np.asarray(inputs["bn1_beta"], f32)
    inv2 = np.asarray(inputs["bn2_gamma"], f32) / np.sqrt(np.asarray(inputs["bn2_var"], f32) + BN_EPS)
    w2f = np.asarray(inputs["conv2_w"], f32) * inv2[:, None, None, None]
    b2f = (np.asarray(inputs["conv2_b"], f32) - np.asarray(inputs["bn2_mean"], f32)) * inv2 \
        + np.asarray(inputs["bn2_beta"], f32)

    # conv1 lhsT [13, 64]: rows (r5, dx) + bias row; cols (jy', c)
    W16 = np.zeros((13, 64), f32)
    for jy in range(2):
        for r5 in range(4):
            dy = r5 - jy
            if 0 <= dy <= 2:
                for dx in range(3):
                    W16[r5 * 3 + dx, jy * 32:(jy + 1) * 32] = w1f[:, 0, dy, dx]
        W16[12, jy * 32:(jy + 1) * 32] = b1f
    W16 = np.ascontiguousarray(W16).astype(BF16)

    # conv1 im2col [13, 1024, 14, 28]: row (r5,dx): xpad[s, 2q+r5, x+dx]; row 12 = 1
    xp = np.zeros((B, 30, 30), f32)
    xp[:, 1:29, 1:29] = x[:, 0]
    xim = np.empty((13, B, 14, 28), f32)
    for r5 in range(4):
        for dx in range(3):
            xim[r5 * 3 + dx] = xp[:, r5:r5 + 27:2, dx:dx + 28]
    xim[12] = 1.0
    xim_cores = [
        np.ascontiguousarray(xim[:, i * B_CORE:(i + 1) * B_CORE]).astype(BF16)
        for i in range(NCORES)
    ]

    # conv2 lhsT [97, 3, 64]: rows (b, ci) + bias row 96 (dy=0 only)
    W2 = np.zeros((97, 3, 64), f32)
    for bi, dx in enumerate([1, 0, 2]):
        for dy in range(3):
            W2[bi * 32:(bi + 1) * 32, dy, :] = w2f[:, :, dy, dx].T
    W2[96, 0, :] = b2f
    W2 = np.ascontiguousarray(W2).astype(BF16)

    # fc moving operand [128, 28, 256]: lane p, K-chunk i, feat f
    fcwf = np.asarray(inputs["fc_w"], f32).reshape(256, 64, 7, 7)  # [f, c, Y, X]
    FCW = np.zeros((128, 28, 256), f32)
    for c in range(64):
        for i in range(28):
            Yq, X = i // 7, i % 7
            FCW[c, i, :] = fcwf[:, c, 2 * Yq, X]
        for i in range(21):
            Yq, X = i // 7, i % 7
            FCW[64 + c, i, :] = fcwf[:, c, 2 * Yq + 1, X]
    FCW = np.ascontiguousarray(FCW).astype(BF16)
    fcb_bf = np.asarray(inputs["fc_b"], f32).reshape(1, 256).astype(BF16)

    id128 = np.eye(128, dtype=f32).astype(BF16)
    zrow = np.zeros((32, 16 * 14), f32).astype(BF16)
    onerow = np.ones((1, 16 * 18 * 14), f32).astype(BF16)

    M, zsigns = _quantum_unitary(np.asarray(inputs["q_params"], np.float64))
    mrt = M.real.T.reshape(2, 128, 2, 128).transpose(1, 0, 2, 3)
    mit = M.imag.T.reshape(2, 128, 2, 128).transpose(1, 0, 2, 3)
    mrt = np.ascontiguousarray(mrt).astype(f32).astype(BF16)
    mit = np.ascontiguousarray(mit).astype(f32).astype(BF16)
    zext = np.ones((DIM, 9), np.float64)
    zext[:, :8] = zsigns
    zext = np.ascontiguousarray(zext.reshape(2, 128, 9).transpose(1, 0, 2)).astype(f32).astype(BF16)

    p1t = np.ascontiguousarray(np.asarray(inputs["p1_w"], f32).T).astype(BF16)  # [8,128]
    p2t = np.ascontiguousarray(np.asarray(inputs["p2_w"], f32).T).astype(BF16)  # [128,64]
    p3t = np.ascontiguousarray(np.asarray(inputs["p3_w"], f32).T).astype(BF16)  # [64,10]

    common = {
        "w16": W16, "w2": W2, "fcw": FCW, "fcb": fcb_bf,
        "id128": id128, "zrow": zrow, "onerow": onerow,
        "mrt": mrt, "mit": mit, "zext": zext,
        "p1t": p1t, "p2t": p2t, "p3t": p3t,
        "pb1": np.asarray(inputs["p1_b"], f32).reshape(128, 1),
        "pb2": np.asarray(inputs["p2_b"], f32).reshape(64, 1),
        "pb3": np.asarray(inputs["p3_b"], f32).reshape(10, 1),
    }
    in_maps = []
    for i in range(NCORES):
        m = dict(common)
        m["xim"] = xim_cores[i]
        in_maps.append(m)
    return in_maps


# ---------------------------------------------------------------- bass program
def _build_bass():
    import concourse.bacc as bacc
    import concourse.mybir as mybir
    import concourse.tile as tile

    dt = mybir.dt
    AF = mybir.ActivationFunctionType
    ALU = mybir.AluOpType

    nc = bacc.Bacc("TRN2", target_bir_lowering=False, debug=False,
                   num_devices=NCORES)
    xim = nc.dram_tensor("xim", [13, B_CORE, 14, 28], dt.bfloat16, kind="ExternalInput")
    w16 = nc.dram_tensor("w16", [13, 64], dt.bfloat16, kind="ExternalInput")
    w2 = nc.dram_tensor("w2", [97, 3, 64], dt.bfloat16, kind="ExternalInput")
    fcw = nc.dram_tensor("fcw", [128, 28, 256], dt.bfloat16, kind="ExternalInput")
    fcb = nc.dram_tensor("fcb", [1, 256], dt.bfloat16, kind="ExternalInput")
    id128 = nc.dram_tensor("id128", [128, 128], dt.bfloat16, kind="ExternalInput")
    zrow = nc.dram_tensor("zrow", [32, 224], dt.bfloat16, kind="ExternalInput")
    onerow = nc.dram_tensor("onerow", [1, 4032], dt.bfloat16, kind="ExternalInput")
    mrt = nc.dram_tensor("mrt", [128, 2, 2, 128], dt.bfloat16, kind="ExternalInput")
    mit = nc.dram_tensor("mit", [128, 2, 2, 128], dt.bfloat16, kind="ExternalInput")
    zext = nc.dram_tensor("zext", [128, 2, 9], dt.bfloat16, kind="ExternalInput")
    p1t = nc.dram_tensor("p1t", [8, 128], dt.bfloat16, kind="ExternalInput")
    p2t = nc.dram_tensor("p2t", [128, 64], dt.bfloat16, kind="ExternalInput")
    p3t = nc.dram_tensor("p3t", [64, 10], dt.bfloat16, kind="ExternalInput")
    pb1 = nc.dram_tensor("pb1", [128, 1], dt.float32, kind="ExternalInput")
    pb2 = nc.dram_tensor("pb2", [64, 1], dt.float32, kind="ExternalInput")
    pb3 = nc.dram_tensor("pb3", [10, 1], dt.float32, kind="ExternalInput")
    out = nc.dram_tensor("out", [10, B_CORE], dt.float32, kind="ExternalOutput")

    with tile.TileContext(nc) as tc:
        with tc.tile_pool(name="singles", bufs=1) as singles:
            w16_sb = singles.tile([13, 64], dt.bfloat16, tag="w16")
            nc.sync.dma_start(out=w16_sb, in_=w16[:, :])
            w2_sb = singles.tile([97, 3, 64], dt.bfloat16, tag="w2")
            nc.sync.dma_start(out=w2_sb, in_=w2[:, :, :])
            fcw_sb = singles.tile([128, 28, 256], dt.bfloat16, tag="fcw")
            fcb_sb = singles.tile([1, 256], dt.bfloat16, tag="fcb")
            id_sb = singles.tile([128, 128], dt.bfloat16, tag="id128")
            mrt_sb = singles.tile([128, 2, 2, 128], dt.bfloat16, tag="mrt")
            mit_sb = singles.tile([128, 2, 2, 128], dt.bfloat16, tag="mit")
            zext_sb = singles.tile([128, 2, 9], dt.bfloat16, tag="zext")
            p1t_sb = singles.tile([8, 128], dt.bfloat16, tag="p1t")
            p2t_sb = singles.tile([128, 64], dt.bfloat16, tag="p2t")
            p3t_sb = singles.tile([64, 10], dt.bfloat16, tag="p3t")
            pb1_sb = singles.tile([128, 1], dt.float32, tag="pb1")
            pb2_sb = singles.tile([64, 1], dt.float32, tag="pb2")
            pb3_sb = singles.tile([10, 1], dt.float32, tag="pb3")
            ones18 = singles.tile([1, 8], dt.bfloat16, tag="ones18")
            nc.vector.memset(ones18, 1.0)
            ones1s = singles.tile([1, 128], dt.bfloat16, tag="ones1s")
            nc.vector.memset(ones1s, 1.0)

            # conv2 input tiles, one per (chunk parity, half):
            # [97=(b,ci)+bias, 16x', 16s, 18y'']; b: 0=center(dx1), 1=L(dx0),
            # 2=R(dx2); lane 96 = ones (bias row).  Persistent pads: center x'
            # slots 0,15 zero; center y'' slots 1,16 zero (L/R copies propagate).
            in2 = [singles.tile([97, HALF, 18, 14], dt.bfloat16, tag=f"in2_{i}",
                                name=f"in2_{i}") for i in range(4)]
            for t in in2:
                nc.gpsimd.dma_start(out=t[0:32, :, 1:2, :], in_=zrow[:, :])
                nc.gpsimd.dma_start(out=t[0:32, :, 16:17, :], in_=zrow[:, :])
                nc.gpsimd.dma_start(
                    out=t[96:97].rearrange("p s y x -> p (s y x)"), in_=onerow[:, :])

            # fc stationary operand [128=(par,c), 128smp, 28i]
            p2full = singles.tile([128, B_CORE, 28], dt.bfloat16, tag="p2full")

            with tc.tile_pool(name="ximp", bufs=2) as ximpool, \
                 tc.tile_pool(name="p1xp", bufs=2) as p1xpool, \
                 tc.tile_pool(name="oddp", bufs=4) as oddpool, \
                 tc.tile_pool(name="scrp", bufs=2) as scrpool, \
                 tc.tile_pool(name="c2xp", bufs=2) as c2xpool, \
                 tc.tile_pool(name="c2yp", bufs=2) as c2ypool, \
                 tc.tile_pool(name="scr2p", bufs=2) as scr2pool, \
                 tc.tile_pool(name="ps1", bufs=4, space="PSUM") as psum1, \
                 tc.tile_pool(name="ps2", bufs=4, space="PSUM") as psum2:
                for ci in range(NCH):
                    xim_sb = ximpool.tile([13, CH, 14, 28], dt.bfloat16, tag="xim_sb")
                    nc.sync.dma_start(out=xim_sb, in_=xim[:, ci * CH:(ci + 1) * CH])
                    if ci == 1:
                        nc.sync.dma_start(out=fcw_sb, in_=fcw[:, :, :])
                        nc.sync.dma_start(out=fcb_sb, in_=fcb[:, :])
                        nc.sync.dma_start(out=id_sb, in_=id128[:, :])
                    elif ci == 2:
                        nc.sync.dma_start(out=mrt_sb, in_=mrt[:, :, :, :])
                        nc.sync.dma_start(out=mit_sb, in_=mit[:, :, :, :])
                        nc.sync.dma_start(out=zext_sb, in_=zext[:, :, :])
                        nc.sync.dma_start(out=p1t_sb, in_=p1t[:, :])
                        nc.sync.dma_start(out=p2t_sb, in_=p2t[:, :])
                        nc.sync.dma_start(out=p3t_sb, in_=p3t[:, :])
                        nc.sync.dma_start(out=pb1_sb, in_=pb1[:, :])
                        nc.sync.dma_start(out=pb2_sb, in_=pb2[:, :])
                        nc.sync.dma_start(out=pb3_sb, in_=pb3[:, :])

                    # ---- conv1: 8 slots (q-block halves x col-packed pairs) ----
                    # p1x layout [128=(half,jy,c), 16smp, 14q(y), 14xp]
                    p1x = p1xpool.tile([128, HALF, 14, 14], dt.bfloat16, tag="p1x")
                    for p in range(8):
                        for h in range(2):
                            q0, q1 = (0, 7) if h == 0 else (7, 14)
                            c1p = psum1.tile([128, 512], dt.float32, tag="c1p")
                            c1v = c1p[:, 0:392].rearrange(
                                "p (s q x) -> p s q x", s=2, q=7, x=28)
                            nc.tensor.matmul(
                                c1v[0:64], w16_sb,
                                xim_sb[:, 2 * p:2 * p + 2, q0:q1, :],
                                start=True, stop=True)
                            nc.tensor.matmul(
                                c1v[64:128], w16_sb,
                                xim_sb[:, HALF + 2 * p:HALF + 2 * p + 2, q0:q1, :],
                                start=True, stop=True, skip_group_check=True)
                            # drain + x-pool: ACT extracts odd-x, DVE stt + relu
                            cph = c1v.rearrange(
                                "p s q (xp two) -> p s q xp two", two=2)
                            oddt = oddpool.tile([128, 2, 7, 14], dt.bfloat16,
                                                tag="oddt")
                            nc.scalar.activation(oddt, cph[:, :, :, :, 1], AF.Copy)
                            nc.vector.scalar_tensor_tensor(
                                p1x[:, 2 * p:2 * p + 2, q0:q1, :],
                                cph[:, :, :, :, 0], 0.0, oddt,
                                ALU.max, ALU.max)

                    # ---- fold into in2 centers (y-interleave, relu fused) ----
                    tA = in2[2 * (ci % 2)]
                    tB = in2[2 * (ci % 2) + 1]
                    scr = scrpool.tile([32, 3, HALF, 14, 14], dt.bfloat16, tag="scr")
                    nc.gpsimd.dma_start(out=scr[:, 0], in_=p1x[32:64])
                    nc.gpsimd.dma_start(out=scr[:, 1], in_=p1x[64:96])
                    nc.gpsimd.dma_start(out=scr[:, 2], in_=p1x[96:128])
                    for t2, in0, in1 in ((tA, p1x[0:32], scr[:, 0]),
                                         (tB, scr[:, 1], scr[:, 2])):
                        nc.vector.tensor_tensor(
                            t2[0:32, :, 2:16, :], in0, in1, ALU.max)
                        # L/R shifted blocks: flat one-element shift (fat runs),
                        # then re-zero the wrapped pad column
                        tf = t2.rearrange("p s y x -> p (s y x)")
                        nc.sync.dma_start(out=tf[32:64, 1:4032], in_=tf[0:32, 0:4031])
                        nc.sync.dma_start(out=tf[64:96, 0:4031], in_=tf[0:32, 1:4032])
                        nc.vector.memset(t2[32:64, :, :, 0:1], 0.0)
                        nc.vector.memset(t2[64:96, :, :, 13:14], 0.0)

                    # ---- conv2: slot-pairs, dy-outer, LDW reuse in 2nd slot ----
                    c2x = c2xpool.tile([128, HALF, 16, 7], dt.bfloat16, tag="c2x")
                    for tp in range(4):
                        cps = [psum2.tile([128, 392], dt.float32, tag="c2p",
                                          name=f"c2p{j}") for j in range(2)]
                        cvs = [c.rearrange("p (s y x) -> p s y x", s=2, y=14, x=14)
                               for c in cps]
                        for dy in range(3):
                            for j in range(2):
                                t = 2 * tp + j
                                rhsA = tA[:, 2 * t:2 * t + 2, dy + 1:dy + 15, :]
                                rhsB = tB[:, 2 * t:2 * t + 2, dy + 1:dy + 15, :]
                                m1 = nc.tensor.matmul(
                                    cvs[j][0:64], w2_sb[:, dy, :], rhsA,
                                    start=(dy == 0), stop=(dy == 2),
                                    skip_group_check=True)
                                m2 = nc.tensor.matmul(
                                    cvs[j][64:128], w2_sb[:, dy, :], rhsB,
                                    start=(dy == 0), stop=(dy == 2),
                                    skip_group_check=True)
                                if j == 1:
                                    m1.ins.ldweights = False
                                    m2.ins.ldweights = False
                        for j in range(2):
                            t = 2 * tp + j
                            cp = cvs[j].rearrange("p s y (xp two) -> p s y xp two",
                                                  two=2)
                            nc.vector.tensor_reduce(
                                c2x[:, 2 * t:2 * t + 2, 0:14, :], cp,
                                mybir.AxisListType.X, ALU.max)

                    # ---- parity y-folds (reduce) + relu splits (ACT) ----
                    # even pixels Y=2Yq: raw row pairs (4Yq, 4Yq+1), Yq 0..3
                    # odd  pixels Y=2Yq+1: raw row pairs (4Yq+2, 4Yq+3), Yq 0..2
                    scr2 = scr2pool.tile([128, HALF, 28], dt.bfloat16, tag="scr2")
                    c2ye = c2ypool.tile([128, HALF, 4, 7], dt.bfloat16, tag="c2ye",
                                        name="c2ye")
                    c2yo = c2ypool.tile([128, HALF, 3, 7], dt.bfloat16, tag="c2yo",
                                        name="c2yo")
                    s0 = ci * CH
                    cyv = c2x.rearrange("p s (yq f) x -> p s yq x f", f=4)
                    nc.vector.tensor_reduce(c2ye, cyv[:, :, :, :, 0:2],
                                            mybir.AxisListType.X, ALU.max)
                    nc.vector.tensor_reduce(c2yo, cyv[:, :, 0:3, :, 2:4],
                                            mybir.AxisListType.X, ALU.max)
                    nc.scalar.activation(p2full[0:64, s0:s0 + HALF, :],
                                         c2ye[0:64], AF.Relu)
                    nc.scalar.activation(scr2[0:64, :, 0:21], c2yo[0:64], AF.Relu)
                    nc.scalar.activation(scr2[64:128, :, 0:28], c2ye[64:128],
                                         AF.Relu)
                    nc.scalar.activation(p2full[64:128, s0 + HALF:s0 + CH, 0:21],
                                         c2yo[64:128], AF.Relu)
                    # parity moves
                    nc.gpsimd.dma_start(out=p2full[64:128, s0:s0 + HALF, 0:21],
                                        in_=scr2[0:64, :, 0:21])
                    nc.gpsimd.dma_start(out=p2full[0:64, s0 + HALF:s0 + CH, :],
                                        in_=scr2[64:128, :, :])
                    # zero the unused K rows once (hi-half i 21..27)
                    if ci == 0:
                        nc.vector.memset(p2full[64:128, :, 21:28], 0.0)

            # ---------------- dense tail ----------------
            with tc.tile_pool(name="tail", bufs=1) as tail, \
                 tc.tile_pool(name="psumT", bufs=1, space="PSUM") as psumT:
                fcp = psumT.tile([128, 256], dt.float32, tag="fcp")
                for i in range(28):
                    nc.tensor.matmul(fcp, p2full[:, :, i], fcw_sb[:, i, :],
                                     start=(i == 0), stop=False)
                nc.tensor.matmul(fcp, ones1s, fcb_sb, start=False, stop=True)
                ftT = tail.tile([128, 256], dt.bfloat16, tag="ftT")
                nc.scalar.activation(ftT, fcp, AF.Tanh)

                feats = tail.tile([128, 2, 128], dt.bfloat16, tag="feats")
                ftp = psumT.tile([128, 2, 128], dt.bfloat16, tag="ftp")
                for mt in range(2):
                    nc.tensor.transpose(ftp[:, mt], ftT[:, mt * 128:(mt + 1) * 128],
                                        id_sb)
                    nc.scalar.activation(feats[:, mt], ftp[:, mt], AF.Copy)

                sq = psumT.tile([128, 4, 128], dt.float32, tag="sq")
                srp = sq[:, 0:2]
                sip = sq[:, 2:4]
                for mt in range(2):
                    for kb in range(2):
                        nc.tensor.matmul(srp[:, mt], mrt_sb[:, kb, mt, :], feats[:, kb],
                                         start=(kb == 0), stop=(kb == 1))
                    for kb in range(2):
                        nc.tensor.matmul(sip[:, mt], mit_sb[:, kb, mt, :], feats[:, kb],
                                         start=(kb == 0), stop=(kb == 1))

                probs = tail.tile([128, 2, 128], dt.bfloat16, tag="probs")
                for mt in range(2):
                    t1 = tail.tile([128, 128], dt.float32, tag=f"sq_r{mt}")
                    nc.scalar.activation(t1, srp[:, mt], AF.Square)
                    t2s = tail.tile([128, 128], dt.float32, tag=f"sq_i{mt}")
                    nc.scalar.activation(t2s, sip[:, mt], AF.Square)
                    nc.vector.tensor_tensor(probs[:, mt], t1, t2s, ALU.add)

                qt = psumT.tile([8, 2, 128], dt.float32, tag="qt")
                qp = qt[:, 0]
                tp = qt[0:1, 1]
                for kb in range(2):
                    nc.tensor.matmul(qp, zext_sb[:, kb, 0:8], probs[:, kb],
                                     start=(kb == 0), stop=(kb == 1))
                for kb in range(2):
                    nc.tensor.matmul(tp, zext_sb[:, kb, 8:9], probs[:, kb],
                                     start=(kb == 0), stop=(kb == 1))

                recip = tail.tile([1, 128], dt.float32, tag="recip")
                nc.vector.reciprocal(recip, tp)
                recip_bf = tail.tile([1, 128], dt.bfloat16, tag="recip_bf")
                nc.vector.tensor_copy(out=recip_bf, in_=recip)
                bc = psumT.tile([8, 128], dt.float32, tag="bc")
                nc.tensor.matmul(bc, ones18, recip_bf, start=True, stop=True)
                bc_sb = tail.tile([8, 128], dt.float32, tag="bc_sb")
                nc.scalar.activation(bc_sb, bc, AF.Copy)

                qn = tail.tile([8, 128], dt.bfloat16, tag="qn")
                nc.vector.tensor_tensor(qn, qp[0:8, :], bc_sb, ALU.mult)

                zp = psumT.tile([128, 3, 128], dt.float32, tag="zp")
                z1p = zp[:, 0]
                z2p = zp[0:64, 1]
                z3p = zp[0:10, 2]
                nc.tensor.matmul(z1p, p1t_sb, qn, start=True, stop=True)
                z1 = tail.tile([128, 128], dt.bfloat16, tag="z1")
                nc.scalar.activation(z1, z1p, AF.Relu, bias=pb1_sb[:, 0:1])

                nc.tensor.matmul(z2p, p2t_sb, z1, start=True, stop=True)
                z2 = tail.tile([64, 128], dt.bfloat16, tag="z2")
                nc.scalar.activation(z2, z2p, AF.Relu, bias=pb2_sb[:, 0:1])

                nc.tensor.matmul(z3p, p3t_sb, z2, start=True, stop=True)
                osb = tail.tile([10, 128], dt.float32, tag="osb")
                nc.vector.tensor_scalar_add(osb, z3p, pb3_sb[:, 0:1])
                nc.sync.dma_start(out=out[:, :], in_=osb)

    nc.finalize()
    return nc


def _get_nc():
    if "nc" not in _CACHE:
        _CACHE["nc"] = _build_bass()
    return _CACHE["nc"]


def kernel(**inputs) -> np.ndarray:
    from concourse.bass_utils import run_bass_kernel_spmd

    in_maps = _host_prep(inputs)
    nc = _get_nc()
    res = run_bass_kernel_spmd(nc, in_maps, core_ids=list(range(NCORES)),
                               trace=bool(_CACHE.get("trace")))
    _CACHE["last_result"] = res
    outs = [r["out"].T for r in res.results]  # each [128, 10]
    return np.ascontiguousarray(np.concatenate(outs, axis=0), dtype=np.float32)


# revision 27
# speedup vs baseline: 1.0692x; 1.0692x over previous
"""AmplitudeQuantumNet Trainium2 kernel (8-core data parallel), v2.

Per core (128 samples, 4 chunks of 32):
  conv1: K=16 im2col (2q+jy rows), M=64 (jy',c), col-packed sample pairs
         -> psum [128]=(A|B); x-pool at drain (dual-psum-AP tt.max)
         -> p1x [128=(half,jy',c), 16smp, 14xp, 16q]
  fold:  3 partition-move DMAs + 2 stt(max,max) ops write relu(pool) straight
         into conv2's input tile (in2 center block, y interleaved, 2x mode)
  conv2: in2 [96=(b,ci), 16x', 32s, 18y''] with L/R blocks built by 2 fat
         contiguous DMAs; 3 accumulating dy-matmuls, col-packed pairs
         -> x-pool at drain -> y-fold -> bias+relu ACT ops that also perform
         the checkerboard pixel split into p2full [128=(par,c), 128s, 28i]
  fc:    activation-stationary: 28 K=128 chunks, moving fcw N=256
         -> psum [128smp, 256feats]; tanh; 2 PE transposes back to
         feats [128=f, 2, 128smp]
  quantum + MLP tail: host-built 256x256 unitary, as before.
"""

import sys

sys.path.insert(0, "/opt/trn_rl_repo")

import numpy as np
import ml_dtypes

BF16 = ml_dtypes.bfloat16

N_QUBITS = 8
Q_DEPTH = 10
DIM = 256
BN_EPS = 1e-5
B = 1024
NCORES = 8
B_CORE = B // NCORES    # 128
NCH = 4
CH = B_CORE // NCH      # 32 samples per chunk
HALF = CH // 2          # 16 (A/B halves)

_CACHE = {}


# ---------------------------------------------------------------- host precompute
def _quantum_unitary(q_params):
    """256x256 complex matrix of the full circuit (H layer + 10x[RX layer + diag])."""
    bits = ((np.arange(DIM)[:, None] >> (N_QUBITS - 1 - np.arange(N_QUBITS))) & 1)
    ph = np.where(np.arange(N_QUBITS) % 2 == 0, 1j, np.exp(1j * np.pi / 4))
    diag = np.prod(np.power(ph[None, :], bits), axis=1)
    cz = np.ones(DIM)
    for i, j in [(0, 1), (2, 3), (4, 5), (6, 7), (1, 2), (3, 4), (5, 6)]:
        cz = cz * ((-1.0) ** (bits[:, i] * bits[:, j]))
    diagc = (diag * cz).astype(np.complex128)

    def app(M, U, w):
        M = M.reshape((2,) * N_QUBITS + (DIM,))
        M = np.moveaxis(M, w, 0)
        M = np.tensordot(U, M, axes=(1, 0))
        M = np.moveaxis(M, 0, w)
        return M.reshape(DIM, DIM)

    M = np.eye(DIM, dtype=np.complex128)
    H = np.array([[1.0, 1.0], [1.0, -1.0]]) / np.sqrt(2.0)
    for w in range(N_QUBITS):
        M = app(M, H, w)
    qw = np.asarray(q_params, np.float64).reshape(Q_DEPTH, N_QUBITS)
    X = np.array([[0.0, 1.0], [1.0, 0.0]])
    I2 = np.eye(2)
    for layer in range(Q_DEPTH):
        for w in range(N_QUBITS):
            t = qw[layer, w]
            U = np.cos(t / 2) * I2 - 1j * np.sin(t / 2) * X
            M = app(M, U, w)
        M = diagc[:, None] * M
    zsigns = (1 - 2 * bits).astype(np.float64)  # [256, 8]
    return M, zsigns


def _host_prep(inputs):
    f32 = np.float32
    x = np.asarray(inputs["x"], f32)  # [1024,1,28,28]

    inv1 = np.asarray(inputs["bn1_gamma"], f32) / np.sqrt(np.asarray(inputs["bn1_var"], f32) + BN_EPS)
    w1f = np.asarray(inputs["conv1_w"], f32) * inv1[:, None, None, None]
    b1f = (np.asarray(inputs["conv1_b"], f32) - np.asarray(inputs["bn1_mean"], f32)) * inv1 \
        + np.asarray(inputs["bn1_beta"], f32)
    inv2 = np.asarray(inputs["bn2_gamma"], f32) / np.sqrt(np.asarray(inputs["bn2_var"], f32) + BN_EPS)
    w2f = np.asarray(inputs["conv2_w"], f32) * inv2[:, None, None, None]
    b2f = (np.asarray(inputs["conv2_b"], f32) - np.asarray(inputs["bn2_mean"], f32)) * inv2 \
        + np.asarray(inputs["bn2_beta"], f32)

    # conv1 lhsT [13, 64]: rows (r5, dx) + bias row; cols (jy', c)
    W16 = np.zeros((13, 64), f32)
    for jy in range(2):
        for r5 in range(4):
            dy = r5 - jy
            if 0 <= dy <= 2:
                for dx in range(3):
                    W16[r5 * 3 + dx, jy * 32:(jy + 1) * 32] = w1f[:, 0, dy, dx]
        W16[12, jy * 32:(jy + 1) * 32] = b1f
    W16 = np.ascontiguousarray(W16).astype(BF16)

    # conv1 im2col [13, 1024, 14, 28]: row (r5,dx): xpad[s, 2q+r5, x+dx]; row 12 = 1
    xp = np.zeros((B, 30, 30), f32)
    xp[:, 1:29, 1:29] = x[:, 0]
    xim = np.empty((13, B, 14, 28), f32)
    for r5 in range(4):
        for dx in range(3):
            xim[r5 * 3 + dx] = xp[:, r5:r5 + 27:2, dx:dx + 28]
    xim[12] = 1.0
    xim_cores = [
        np.ascontiguousarray(xim[:, i * B_CORE:(i + 1) * B_CORE]).astype(BF16)
        for i in range(NCORES)
    ]

    # conv2 lhsT [97, 3, 64]: rows (b, ci) + bias row 96 (dy=0 only)
    W2 = np.zeros((97, 3, 64), f32)
    for bi, dx in enumerate([1, 0, 2]):
        for dy in range(3):
            W2[bi * 32:(bi + 1) * 32, dy, :] = w2f[:, :, dy, dx].T
    W2[96, 0, :] = b2f
    W2 = np.ascontiguousarray(W2).astype(BF16)

    # fc moving operand [128, 28, 256]: lane p, K-chunk i, feat f
    fcwf = np.asarray(inputs["fc_w"], f32).reshape(256, 64, 7, 7)  # [f, c, Y, X]
    FCW = np.zeros((128, 28, 256), f32)
    for c in range(64):
        for i in range(28):
            Yq, X = i // 7, i % 7
            FCW[c, i, :] = fcwf[:, c, 2 * Yq, X]
        for i in range(21):
            Yq, X = i // 7, i % 7
            FCW[64 + c, i, :] = fcwf[:, c, 2 * Yq + 1, X]
    FCW = np.ascontiguousarray(FCW).astype(BF16)
    fcb_bf = np.asarray(inputs["fc_b"], f32).reshape(1, 256).astype(BF16)

    id128 = np.eye(128, dtype=f32).astype(BF16)
    zrow = np.zeros((32, 16 * 14), f32).astype(BF16)
    onerow = np.ones((1, 16 * 18 * 14), f32).astype(BF16)

    M, zsigns = _quantum_unitary(np.asarray(inputs["q_params"], np.float64))
    mrt = M.real.T.reshape(2, 128, 2, 128).transpose(1, 0, 2, 3)
    mit = M.imag.T.reshape(2, 128, 2, 128).transpose(1, 0, 2, 3)
    mrt = np.ascontiguousarray(mrt).astype(f32).astype(BF16)
    mit = np.ascontiguousarray(mit).astype(f32).astype(BF16)
    zext = np.ones((DIM, 9), np.float64)
    zext[:, :8] = zsigns
    zext = np.ascontiguousarray(zext.reshape(2, 128, 9).transpose(1, 0, 2)).astype(f32).astype(BF16)

    p1t = np.ascontiguousarray(np.asarray(inputs["p1_w"], f32).T).astype(BF16)  # [8,128]
    p2t = np.ascontiguousarray(np.asarray(inputs["p2_w"], f32).T).astype(BF16)  # [128,64]
    p3t = np.ascontiguousarray(np.asarray(inputs["p3_w"], f32).T).astype(BF16)  # [64,10]

    common = {
        "w16": W16, "w2": W2, "fcw": FCW, "fcb": fcb_bf,
        "id128": id128, "zrow": zrow, "onerow": onerow,
        "mrt": mrt, "mit": mit, "zext": zext,
        "p1t": p1t, "p2t": p2t, "p3t": p3t,
        "pb1": np.asarray(inputs["p1_b"], f32).reshape(128, 1),
        "pb2": np.asarray(inputs["p2_b"], f32).reshape(64, 1),
        "pb3": np.asarray(inputs["p3_b"], f32).reshape(10, 1),
    }
    in_maps = []
    for i in range(NCORES):
        m = dict(common)
        m["xim"] = xim_cores[i]
        in_maps.append(m)
    return in_maps


# ---------------------------------------------------------------- bass program
def _build_bass():
    import concourse.bacc as bacc
    import concourse.mybir as mybir
    import concourse.tile as tile

    dt = mybir.dt
    AF = mybir.ActivationFunctionType
    ALU = mybir.AluOpType

    nc = bacc.Bacc("TRN2", target_bir_lowering=False, debug=False,
                   num_devices=NCORES)
    xim = nc.dram_tensor("xim", [13, B_CORE, 14, 28], dt.bfloat16, kind="ExternalInput")
    w16 = nc.dram_tensor("w16", [13, 64], dt.bfloat16, kind="ExternalInput")
    w2 = nc.dram_tensor("w2", [97, 3, 64], dt.bfloat16, kind="ExternalInput")
    fcw = nc.dram_tensor("fcw", [128, 28, 256], dt.bfloat16, kind="ExternalInput")
    fcb = nc.dram_tensor("fcb", [1, 256], dt.bfloat16, kind="ExternalInput")
    id128 = nc.dram_tensor("id128", [128, 128], dt.bfloat16, kind="ExternalInput")
    zrow = nc.dram_tensor("zrow", [32, 224], dt.bfloat16, kind="ExternalInput")
    onerow = nc.dram_tensor("onerow", [1, 4032], dt.bfloat16, kind="ExternalInput")
    mrt = nc.dram_tensor("mrt", [128, 2, 2, 128], dt.bfloat16, kind="ExternalInput")
    mit = nc.dram_tensor("mit", [128, 2, 2, 128], dt.bfloat16, kind="ExternalInput")
    zext = nc.dram_tensor("zext", [128, 2, 9], dt.bfloat16, kind="ExternalInput")
    p1t = nc.dram_tensor("p1t", [8, 128], dt.bfloat16, kind="ExternalInput")
    p2t = nc.dram_tensor("p2t", [128, 64], dt.bfloat16, kind="ExternalInput")
    p3t = nc.dram_tensor("p3t", [64, 10], dt.bfloat16, kind="ExternalInput")
    pb1 = nc.dram_tensor("pb1", [128, 1], dt.float32, kind="ExternalInput")
    pb2 = nc.dram_tensor("pb2", [64, 1], dt.float32, kind="ExternalInput")
    pb3 = nc.dram_tensor("pb3", [10, 1], dt.float32, kind="ExternalInput")
    out = nc.dram_tensor("out", [10, B_CORE], dt.float32, kind="ExternalOutput")

    with tile.TileContext(nc) as tc:
        with tc.tile_pool(name="singles", bufs=1) as singles:
            w16_sb = singles.tile([13, 64], dt.bfloat16, tag="w16")
            nc.sync.dma_start(out=w16_sb, in_=w16[:, :])
            w2_sb = singles.tile([97, 3, 64], dt.bfloat16, tag="w2")
            nc.sync.dma_start(out=w2_sb, in_=w2[:, :, :])
            fcw_sb = singles.tile([128, 28, 256], dt.bfloat16, tag="fcw")
            fcb_sb = singles.tile([1, 256], dt.bfloat16, tag="fcb")
            id_sb = singles.tile([128, 128], dt.bfloat16, tag="id128")
            mrt_sb = singles.tile([128, 2, 2, 128], dt.bfloat16, tag="mrt")
            mit_sb = singles.tile([128, 2, 2, 128], dt.bfloat16, tag="mit")
            zext_sb = singles.tile([128, 2, 9], dt.bfloat16, tag="zext")
            p1t_sb = singles.tile([8, 128], dt.bfloat16, tag="p1t")
            p2t_sb = singles.tile([128, 64], dt.bfloat16, tag="p2t")
            p3t_sb = singles.tile([64, 10], dt.bfloat16, tag="p3t")
            pb1_sb = singles.tile([128, 1], dt.float32, tag="pb1")
            pb2_sb = singles.tile([64, 1], dt.float32, tag="pb2")
            pb3_sb = singles.tile([10, 1], dt.float32, tag="pb3")
            ones18 = singles.tile([1, 8], dt.bfloat16, tag="ones18")
            nc.vector.memset(ones18, 1.0)
            ones1s = singles.tile([1, 128], dt.bfloat16, tag="ones1s")
            nc.vector.memset(ones1s, 1.0)

            # conv2 input tiles, one per (chunk parity, half):
            # [97=(b,ci)+bias, 16x', 16s, 18y'']; b: 0=center(dx1), 1=L(dx0),
            # 2=R(dx2); lane 96 = ones (bias row).  Persistent pads: center x'
            # slots 0,15 zero; center y'' slots 1,16 zero (L/R copies propagate).
            in2 = [singles.tile([97, HALF, 18, 14], dt.bfloat16, tag=f"in2_{i}",
                                name=f"in2_{i}") for i in range(4)]
            for t in in2:
                nc.gpsimd.dma_start(out=t[0:32, :, 1:2, :], in_=zrow[:, :])
                nc.gpsimd.dma_start(out=t[0:32, :, 16:17, :], in_=zrow[:, :])
                nc.gpsimd.dma_start(
                    out=t[96:97].rearrange("p s y x -> p (s y x)"), in_=onerow[:, :])

            # fc stationary operand [128=(par,c), 128smp, 28i]
            p2full = singles.tile([128, B_CORE, 28], dt.bfloat16, tag="p2full")

            with tc.tile_pool(name="ximp", bufs=2) as ximpool, \
                 tc.tile_pool(name="p1xp", bufs=2) as p1xpool, \
                 tc.tile_pool(name="oddp", bufs=4) as oddpool, \
                 tc.tile_pool(name="scrp", bufs=2) as scrpool, \
                 tc.tile_pool(name="c2xp", bufs=2) as c2xpool, \
                 tc.tile_pool(name="c2yp", bufs=2) as c2ypool, \
                 tc.tile_pool(name="scr2p", bufs=2) as scr2pool, \
                 tc.tile_pool(name="ps1", bufs=4, space="PSUM") as psum1, \
                 tc.tile_pool(name="ps2", bufs=4, space="PSUM") as psum2:
                for ci in range(NCH):
                    xim_sb = ximpool.tile([13, CH, 14, 28], dt.bfloat16, tag="xim_sb")
                    nc.sync.dma_start(out=xim_sb, in_=xim[:, ci * CH:(ci + 1) * CH])
                    if ci == 1:
                        nc.gpsimd.dma_start(out=fcw_sb, in_=fcw[:, :, :])
                        nc.gpsimd.dma_start(out=fcb_sb, in_=fcb[:, :])
                        nc.gpsimd.dma_start(out=id_sb, in_=id128[:, :])
                    elif ci == 2:
                        nc.gpsimd.dma_start(out=mrt_sb, in_=mrt[:, :, :, :])
                        nc.gpsimd.dma_start(out=mit_sb, in_=mit[:, :, :, :])
                        nc.gpsimd.dma_start(out=zext_sb, in_=zext[:, :, :])
                        nc.gpsimd.dma_start(out=p1t_sb, in_=p1t[:, :])
                        nc.gpsimd.dma_start(out=p2t_sb, in_=p2t[:, :])
                        nc.gpsimd.dma_start(out=p3t_sb, in_=p3t[:, :])
                        nc.gpsimd.dma_start(out=pb1_sb, in_=pb1[:, :])
                        nc.gpsimd.dma_start(out=pb2_sb, in_=pb2[:, :])
                        nc.gpsimd.dma_start(out=pb3_sb, in_=pb3[:, :])

                    # ---- conv1: 8 slots (q-block halves x col-packed pairs) ----
                    # p1x layout [128=(half,jy,c), 16smp, 14q(y), 14xp]
                    p1x = p1xpool.tile([128, HALF, 14, 14], dt.bfloat16, tag="p1x")
                    for p in range(8):
                        for h in range(2):
                            q0, q1 = (0, 7) if h == 0 else (7, 14)
                            c1p = psum1.tile([128, 512], dt.float32, tag="c1p")
                            c1v = c1p[:, 0:392].rearrange(
                                "p (s q x) -> p s q x", s=2, q=7, x=28)
                            nc.tensor.matmul(
                                c1v[0:64], w16_sb,
                                xim_sb[:, 2 * p:2 * p + 2, q0:q1, :],
                                start=True, stop=True)
                            nc.tensor.matmul(
                                c1v[64:128], w16_sb,
                                xim_sb[:, HALF + 2 * p:HALF + 2 * p + 2, q0:q1, :],
                                start=True, stop=True, skip_group_check=True)
                            # drain + x-pool: ACT extracts odd-x, DVE stt + relu
                            cph = c1v.rearrange(
                                "p s q (xp two) -> p s q xp two", two=2)
                            oddt = oddpool.tile([128, 2, 7, 14], dt.bfloat16,
                                                tag="oddt")
                            nc.scalar.activation(oddt, cph[:, :, :, :, 1], AF.Copy)
                            nc.vector.scalar_tensor_tensor(
                                p1x[:, 2 * p:2 * p + 2, q0:q1, :],
                                cph[:, :, :, :, 0], 0.0, oddt,
                                ALU.max, ALU.max)

                    # ---- fold into in2 centers (y-interleave, relu fused) ----
                    tA = in2[2 * (ci % 2)]
                    tB = in2[2 * (ci % 2) + 1]
                    scr = scrpool.tile([32, 3, HALF, 14, 14], dt.bfloat16, tag="scr")
                    nc.gpsimd.dma_start(out=scr[:, 0], in_=p1x[32:64])
                    nc.gpsimd.dma_start(out=scr[:, 1], in_=p1x[64:96])
                    nc.gpsimd.dma_start(out=scr[:, 2], in_=p1x[96:128])
                    for t2, in0, in1 in ((tA, p1x[0:32], scr[:, 0]),
                                         (tB, scr[:, 1], scr[:, 2])):
                        nc.vector.tensor_tensor(
                            t2[0:32, :, 2:16, :], in0, in1, ALU.max)
                        # L/R shifted blocks: flat one-element shift (fat runs),
                        # then re-zero the wrapped pad column
                        tf = t2.rearrange("p s y x -> p (s y x)")
                        nc.sync.dma_start(out=tf[32:64, 1:4032], in_=tf[0:32, 0:4031])
                        nc.sync.dma_start(out=tf[64:96, 0:4031], in_=tf[0:32, 1:4032])
                        nc.vector.memset(t2[32:64, :, :, 0:1], 0.0)
                        nc.vector.memset(t2[64:96, :, :, 13:14], 0.0)

                    # ---- conv2: 8 slots of col-packed 3-dy MM chains ----
                    c2x = c2xpool.tile([128, HALF, 16, 7], dt.bfloat16, tag="c2x")
                    for t in range(8):
                        c2p = psum2.tile([128, 392], dt.float32, tag="c2p")
                        c2v = c2p.rearrange("p (s y x) -> p s y x", s=2, y=14, x=14)
                        for dy in range(3):
                            rhsA = tA[:, 2 * t:2 * t + 2, dy + 1:dy + 15, :]
                            rhsB = tB[:, 2 * t:2 * t + 2, dy + 1:dy + 15, :]
                            nc.tensor.matmul(c2v[0:64], w2_sb[:, dy, :], rhsA,
                                             start=(dy == 0), stop=(dy == 2))
                            nc.tensor.matmul(c2v[64:128], w2_sb[:, dy, :], rhsB,
                                             start=(dy == 0), stop=(dy == 2),
                                             skip_group_check=True)
                        cp = c2v.rearrange("p s y (xp two) -> p s y xp two", two=2)
                        nc.vector.tensor_reduce(
                            c2x[:, 2 * t:2 * t + 2, 0:14, :], cp,
                            mybir.AxisListType.X, ALU.max)

                    # ---- parity y-folds (reduce) + relu splits (ACT) ----
                    # even pixels Y=2Yq: raw row pairs (4Yq, 4Yq+1), Yq 0..3
                    # odd  pixels Y=2Yq+1: raw row pairs (4Yq+2, 4Yq+3), Yq 0..2
                    scr2 = scr2pool.tile([128, HALF, 28], dt.bfloat16, tag="scr2")
                    c2ye = c2ypool.tile([128, HALF, 4, 7], dt.bfloat16, tag="c2ye",
                                        name="c2ye")
                    c2yo = c2ypool.tile([128, HALF, 3, 7], dt.bfloat16, tag="c2yo",
                                        name="c2yo")
                    s0 = ci * CH
                    cyv = c2x.rearrange("p s (yq f) x -> p s yq x f", f=4)
                    nc.vector.tensor_reduce(c2ye, cyv[:, :, :, :, 0:2],
                                            mybir.AxisListType.X, ALU.max)
                    nc.vector.tensor_reduce(c2yo, cyv[:, :, 0:3, :, 2:4],
                                            mybir.AxisListType.X, ALU.max)
                    nc.scalar.activation(p2full[0:64, s0:s0 + HALF, :],
                                         c2ye[0:64], AF.Relu)
                    nc.scalar.activation(scr2[0:64, :, 0:21], c2yo[0:64], AF.Relu)
                    nc.scalar.activation(scr2[64:128, :, 0:28], c2ye[64:128],
                                         AF.Relu)
                    nc.scalar.activation(p2full[64:128, s0 + HALF:s0 + CH, 0:21],
                                         c2yo[64:128], AF.Relu)
                    # parity moves
                    nc.gpsimd.dma_start(out=p2full[64:128, s0:s0 + HALF, 0:21],
                                        in_=scr2[0:64, :, 0:21])
                    nc.gpsimd.dma_start(out=p2full[0:64, s0 + HALF:s0 + CH, :],
                                        in_=scr2[64:128, :, :])
                    # zero the unused K rows once (hi-half i 21..27)
                    if ci == 0:
                        nc.vector.memset(p2full[64:128, :, 21:28], 0.0)

            # ---------------- dense tail ----------------
            with tc.tile_pool(name="tail", bufs=1) as tail, \
                 tc.tile_pool(name="psumT", bufs=1, space="PSUM") as psumT:
                fcp = psumT.tile([128, 256], dt.float32, tag="fcp")
                for i in range(28):
                    nc.tensor.matmul(fcp, p2full[:, :, i], fcw_sb[:, i, :],
                                     start=(i == 0), stop=False)
                nc.tensor.matmul(fcp, ones1s, fcb_sb, start=False, stop=True)
                ftT = tail.tile([128, 256], dt.bfloat16, tag="ftT")
                nc.scalar.activation(ftT, fcp, AF.Tanh)

                feats = tail.tile([128, 2, 128], dt.bfloat16, tag="feats")
                ftp = psumT.tile([128, 2, 128], dt.bfloat16, tag="ftp")
                for mt in range(2):
                    nc.tensor.transpose(ftp[:, mt], ftT[:, mt * 128:(mt + 1) * 128],
                                        id_sb)
                    nc.scalar.activation(feats[:, mt], ftp[:, mt], AF.Copy)

                sq = psumT.tile([128, 4, 128], dt.float32, tag="sq")
                srp = sq[:, 0:2]
                sip = sq[:, 2:4]
                for mt in range(2):
                    for kb in range(2):
                        nc.tensor.matmul(srp[:, mt], mrt_sb[:, kb, mt, :], feats[:, kb],
                                         start=(kb == 0), stop=(kb == 1))
                    for kb in range(2):
                        nc.tensor.matmul(sip[:, mt], mit_sb[:, kb, mt, :], feats[:, kb],
                                         start=(kb == 0), stop=(kb == 1))

                probs = tail.tile([128, 2, 128], dt.bfloat16, tag="probs")
                for mt in range(2):
                    t1 = tail.tile([128, 128], dt.float32, tag=f"sq_r{mt}")
                    nc.scalar.activation(t1, srp[:, mt], AF.Square)
                    t2s = tail.tile([128, 128], dt.float32, tag=f"sq_i{mt}")
                    nc.scalar.activation(t2s, sip[:, mt], AF.Square)
                    nc.vector.tensor_tensor(probs[:, mt], t1, t2s, ALU.add)

                qt = psumT.tile([8, 2, 128], dt.float32, tag="qt")
                qp = qt[:, 0]
                tp = qt[0:1, 1]
                for kb in range(2):
                    nc.tensor.matmul(qp, zext_sb[:, kb, 0:8], probs[:, kb],
                                     start=(kb == 0), stop=(kb == 1))
                for kb in range(2):
                    nc.tensor.matmul(tp, zext_sb[:, kb, 8:9], probs[:, kb],
                                     start=(kb == 0), stop=(kb == 1))

                recip = tail.tile([1, 128], dt.float32, tag="recip")
                nc.vector.reciprocal(recip, tp)
                recip_bf = tail.tile([1, 128], dt.bfloat16, tag="recip_bf")
                nc.vector.tensor_copy(out=recip_bf, in_=recip)
                bc = psumT.tile([8, 128], dt.float32, tag="bc")
                nc.tensor.matmul(bc, ones18, recip_bf, start=True, stop=True)
                bc_sb = tail.tile([8, 128], dt.float32, tag="bc_sb")
                nc.scalar.activation(bc_sb, bc, AF.Copy)

                qn = tail.tile([8, 128], dt.bfloat16, tag="qn")
                nc.vector.tensor_tensor(qn, qp[0:8, :], bc_sb, ALU.mult)

                zp = psumT.tile([128, 3, 128], dt.float32, tag="zp")
                z1p = zp[:, 0]
                z2p = zp[0:64, 1]
                z3p = zp[0:10, 2]
                nc.tensor.matmul(z1p, p1t_sb, qn, start=True, stop=True)
                z1 = tail.tile([128, 128], dt.bfloat16, tag="z1")
                nc.scalar.activation(z1, z1p, AF.Relu, bias=pb1_sb[:, 0:1])

                nc.tensor.matmul(z2p, p2t_sb, z1, start=True, stop=True)
                z2 = tail.tile([64, 128], dt.bfloat16, tag="z2")
                nc.scalar.activation(z2, z2p, AF.Relu, bias=pb2_sb[:, 0:1])

                nc.tensor.matmul(z3p, p3t_sb, z2, start=True, stop=True)
                osb = tail.tile([10, 128], dt.float32, tag="osb")
                nc.vector.tensor_scalar_add(osb, z3p, pb3_sb[:, 0:1])
                nc.sync.dma_start(out=out[:, :], in_=osb)

    nc.finalize()
    return nc


def _get_nc():
    if "nc" not in _CACHE:
        _CACHE["nc"] = _build_bass()
    return _CACHE["nc"]


def kernel(**inputs) -> np.ndarray:
    from concourse.bass_utils import run_bass_kernel_spmd

    in_maps = _host_prep(inputs)
    nc = _get_nc()
    res = run_bass_kernel_spmd(nc, in_maps, core_ids=list(range(NCORES)),
                               trace=bool(_CACHE.get("trace")))
    _CACHE["last_result"] = res
    outs = [r["out"].T for r in res.results]  # each [128, 10]
    return np.ascontiguousarray(np.concatenate(outs, axis=0), dtype=np.float32)


# revision 31
# speedup vs baseline: 1.0698x; 1.0005x over previous
"""AmplitudeQuantumNet Trainium2 kernel (8-core data parallel), v2.

Per core (128 samples, 4 chunks of 32):
  conv1: K=16 im2col (2q+jy rows), M=64 (jy',c), col-packed sample pairs
         -> psum [128]=(A|B); x-pool at drain (dual-psum-AP tt.max)
         -> p1x [128=(half,jy',c), 16smp, 14xp, 16q]
  fold:  3 partition-move DMAs + 2 stt(max,max) ops write relu(pool) straight
         into conv2's input tile (in2 center block, y interleaved, 2x mode)
  conv2: in2 [96=(b,ci), 16x', 32s, 18y''] with L/R blocks built by 2 fat
         contiguous DMAs; 3 accumulating dy-matmuls, col-packed pairs
         -> x-pool at drain -> y-fold -> bias+relu ACT ops that also perform
         the checkerboard pixel split into p2full [128=(par,c), 128s, 28i]
  fc:    activation-stationary: 28 K=128 chunks, moving fcw N=256
         -> psum [128smp, 256feats]; tanh; 2 PE transposes back to
         feats [128=f, 2, 128smp]
  quantum + MLP tail: host-built 256x256 unitary, as before.
"""

import sys

sys.path.insert(0, "/opt/trn_rl_repo")

import numpy as np
import ml_dtypes

BF16 = ml_dtypes.bfloat16

N_QUBITS = 8
Q_DEPTH = 10
DIM = 256
BN_EPS = 1e-5
B = 1024
NCORES = 8
B_CORE = B // NCORES    # 128
NCH = 4
CH = B_CORE // NCH      # 32 samples per chunk
HALF = CH // 2          # 16 (A/B halves)

_CACHE = {}


# ---------------------------------------------------------------- host precompute
def _quantum_unitary(q_params):
    """256x256 complex matrix of the full circuit (H layer + 10x[RX layer + diag])."""
    bits = ((np.arange(DIM)[:, None] >> (N_QUBITS - 1 - np.arange(N_QUBITS))) & 1)
    ph = np.where(np.arange(N_QUBITS) % 2 == 0, 1j, np.exp(1j * np.pi / 4))
    diag = np.prod(np.power(ph[None, :], bits), axis=1)
    cz = np.ones(DIM)
    for i, j in [(0, 1), (2, 3), (4, 5), (6, 7), (1, 2), (3, 4), (5, 6)]:
        cz = cz * ((-1.0) ** (bits[:, i] * bits[:, j]))
    diagc = (diag * cz).astype(np.complex128)

    def app(M, U, w):
        M = M.reshape((2,) * N_QUBITS + (DIM,))
        M = np.moveaxis(M, w, 0)
        M = np.tensordot(U, M, axes=(1, 0))
        M = np.moveaxis(M, 0, w)
        return M.reshape(DIM, DIM)

    M = np.eye(DIM, dtype=np.complex128)
    H = np.array([[1.0, 1.0], [1.0, -1.0]]) / np.sqrt(2.0)
    for w in range(N_QUBITS):
        M = app(M, H, w)
    qw = np.asarray(q_params, np.float64).reshape(Q_DEPTH, N_QUBITS)
    X = np.array([[0.0, 1.0], [1.0, 0.0]])
    I2 = np.eye(2)
    for layer in range(Q_DEPTH):
        for w in range(N_QUBITS):
            t = qw[layer, w]
            U = np.cos(t / 2) * I2 - 1j * np.sin(t / 2) * X
            M = app(M, U, w)
        M = diagc[:, None] * M
    zsigns = (1 - 2 * bits).astype(np.float64)  # [256, 8]
    return M, zsigns


def _host_prep(inputs):
    f32 = np.float32
    x = np.asarray(inputs["x"], f32)  # [1024,1,28,28]

    inv1 = np.asarray(inputs["bn1_gamma"], f32) / np.sqrt(np.asarray(inputs["bn1_var"], f32) + BN_EPS)
    w1f = np.asarray(inputs["conv1_w"], f32) * inv1[:, None, None, None]
    b1f = (np.asarray(inputs["conv1_b"], f32) - np.asarray(inputs["bn1_mean"], f32)) * inv1 \
        + np.asarray(inputs["bn1_beta"], f32)
    inv2 = np.asarray(inputs["bn2_gamma"], f32) / np.sqrt(np.asarray(inputs["bn2_var"], f32) + BN_EPS)
    w2f = np.asarray(inputs["conv2_w"], f32) * inv2[:, None, None, None]
    b2f = (np.asarray(inputs["conv2_b"], f32) - np.asarray(inputs["bn2_mean"], f32)) * inv2 \
        + np.asarray(inputs["bn2_beta"], f32)

    # conv1 lhsT [13, 64]: rows (r5, dx) + bias row; cols (jy', c)
    W16 = np.zeros((13, 64), f32)
    for jy in range(2):
        for r5 in range(4):
            dy = r5 - jy
            if 0 <= dy <= 2:
                for dx in range(3):
                    W16[r5 * 3 + dx, jy * 32:(jy + 1) * 32] = w1f[:, 0, dy, dx]
        W16[12, jy * 32:(jy + 1) * 32] = b1f
    W16 = np.ascontiguousarray(W16).astype(BF16)

    # conv1 im2col [13, 1024, 14, 28]: row (r5,dx): xpad[s, 2q+r5, x+dx]; row 12 = 1
    xp = np.zeros((B, 30, 30), f32)
    xp[:, 1:29, 1:29] = x[:, 0]
    xim = np.empty((13, B, 14, 28), f32)
    for r5 in range(4):
        for dx in range(3):
            xim[r5 * 3 + dx] = xp[:, r5:r5 + 27:2, dx:dx + 28]
    xim[12] = 1.0
    xim_cores = [
        np.ascontiguousarray(xim[:, i * B_CORE:(i + 1) * B_CORE]).astype(BF16)
        for i in range(NCORES)
    ]

    # conv2 lhsT [97, 3, 64]: rows (b, ci) + bias row 96 (dy=0 only)
    W2 = np.zeros((97, 3, 64), f32)
    for bi, dx in enumerate([1, 0, 2]):
        for dy in range(3):
            W2[bi * 32:(bi + 1) * 32, dy, :] = w2f[:, :, dy, dx].T
    W2[96, 0, :] = b2f
    W2 = np.ascontiguousarray(W2).astype(BF16)

    # fc moving operand [128, 28, 256]: lane p, K-chunk i, feat f
    fcwf = np.asarray(inputs["fc_w"], f32).reshape(256, 64, 7, 7)  # [f, c, Y, X]
    FCW = np.zeros((128, 28, 256), f32)
    for c in range(64):
        for i in range(28):
            Yq, X = i // 7, i % 7
            FCW[c, i, :] = fcwf[:, c, 2 * Yq, X]
        for i in range(21):
            Yq, X = i // 7, i % 7
            FCW[64 + c, i, :] = fcwf[:, c, 2 * Yq + 1, X]
    FCW = np.ascontiguousarray(FCW).astype(BF16)
    fcb_bf = np.asarray(inputs["fc_b"], f32).reshape(1, 256).astype(BF16)

    id128 = np.eye(128, dtype=f32).astype(BF16)
    zrow = np.zeros((32, 16 * 14), f32).astype(BF16)
    onerow = np.ones((1, 16 * 18 * 14), f32).astype(BF16)

    M, zsigns = _quantum_unitary(np.asarray(inputs["q_params"], np.float64))
    mrt = M.real.T.reshape(2, 128, 2, 128).transpose(1, 0, 2, 3)
    mit = M.imag.T.reshape(2, 128, 2, 128).transpose(1, 0, 2, 3)
    mrt = np.ascontiguousarray(mrt).astype(f32).astype(BF16)
    mit = np.ascontiguousarray(mit).astype(f32).astype(BF16)
    zext = np.ones((DIM, 9), np.float64)
    zext[:, :8] = zsigns
    zext = np.ascontiguousarray(zext.reshape(2, 128, 9).transpose(1, 0, 2)).astype(f32).astype(BF16)

    p1t = np.ascontiguousarray(np.asarray(inputs["p1_w"], f32).T).astype(BF16)  # [8,128]
    p2t = np.ascontiguousarray(np.asarray(inputs["p2_w"], f32).T).astype(BF16)  # [128,64]
    p3t = np.ascontiguousarray(np.asarray(inputs["p3_w"], f32).T).astype(BF16)  # [64,10]

    common = {
        "w16": W16, "w2": W2, "fcw": FCW, "fcb": fcb_bf,
        "id128": id128, "zrow": zrow, "onerow": onerow,
        "mrt": mrt, "mit": mit, "zext": zext,
        "p1t": p1t, "p2t": p2t, "p3t": p3t,
        "pb1": np.asarray(inputs["p1_b"], f32).reshape(128, 1),
        "pb2": np.asarray(inputs["p2_b"], f32).reshape(64, 1),
        "pb3": np.asarray(inputs["p3_b"], f32).reshape(10, 1),
    }
    in_maps = []
    for i in range(NCORES):
        m = dict(common)
        m["xim"] = xim_cores[i]
        in_maps.append(m)
    return in_maps


# ---------------------------------------------------------------- bass program
def _build_bass():
    import concourse.bacc as bacc
    import concourse.mybir as mybir
    import concourse.tile as tile

    dt = mybir.dt
    AF = mybir.ActivationFunctionType
    ALU = mybir.AluOpType

    nc = bacc.Bacc("TRN2", target_bir_lowering=False, debug=False,
                   num_devices=NCORES)
    xim = nc.dram_tensor("xim", [13, B_CORE, 14, 28], dt.bfloat16, kind="ExternalInput")
    w16 = nc.dram_tensor("w16", [13, 64], dt.bfloat16, kind="ExternalInput")
    w2 = nc.dram_tensor("w2", [97, 3, 64], dt.bfloat16, kind="ExternalInput")
    fcw = nc.dram_tensor("fcw", [128, 28, 256], dt.bfloat16, kind="ExternalInput")
    fcb = nc.dram_tensor("fcb", [1, 256], dt.bfloat16, kind="ExternalInput")
    id128 = nc.dram_tensor("id128", [128, 128], dt.bfloat16, kind="ExternalInput")
    zrow = nc.dram_tensor("zrow", [32, 224], dt.bfloat16, kind="ExternalInput")
    onerow = nc.dram_tensor("onerow", [1, 4032], dt.bfloat16, kind="ExternalInput")
    mrt = nc.dram_tensor("mrt", [128, 2, 2, 128], dt.bfloat16, kind="ExternalInput")
    mit = nc.dram_tensor("mit", [128, 2, 2, 128], dt.bfloat16, kind="ExternalInput")
    zext = nc.dram_tensor("zext", [128, 2, 9], dt.bfloat16, kind="ExternalInput")
    p1t = nc.dram_tensor("p1t", [8, 128], dt.bfloat16, kind="ExternalInput")
    p2t = nc.dram_tensor("p2t", [128, 64], dt.bfloat16, kind="ExternalInput")
    p3t = nc.dram_tensor("p3t", [64, 10], dt.bfloat16, kind="ExternalInput")
    pb1 = nc.dram_tensor("pb1", [128, 1], dt.float32, kind="ExternalInput")
    pb2 = nc.dram_tensor("pb2", [64, 1], dt.float32, kind="ExternalInput")
    pb3 = nc.dram_tensor("pb3", [10, 1], dt.float32, kind="ExternalInput")
    out = nc.dram_tensor("out", [10, B_CORE], dt.float32, kind="ExternalOutput")

    with tile.TileContext(nc) as tc:
        with tc.tile_pool(name="singles", bufs=1) as singles:
            w16_sb = singles.tile([13, 64], dt.bfloat16, tag="w16")
            nc.sync.dma_start(out=w16_sb, in_=w16[:, :])
            w2_sb = singles.tile([97, 3, 64], dt.bfloat16, tag="w2")
            nc.sync.dma_start(out=w2_sb, in_=w2[:, :, :])
            fcw_sb = singles.tile([128, 28, 256], dt.bfloat16, tag="fcw")
            fcb_sb = singles.tile([1, 256], dt.bfloat16, tag="fcb")
            id_sb = singles.tile([128, 128], dt.bfloat16, tag="id128")
            mrt_sb = singles.tile([128, 2, 2, 128], dt.bfloat16, tag="mrt")
            mit_sb = singles.tile([128, 2, 2, 128], dt.bfloat16, tag="mit")
            zext_sb = singles.tile([128, 2, 9], dt.bfloat16, tag="zext")
            p1t_sb = singles.tile([8, 128], dt.bfloat16, tag="p1t")
            p2t_sb = singles.tile([128, 64], dt.bfloat16, tag="p2t")
            p3t_sb = singles.tile([64, 10], dt.bfloat16, tag="p3t")
            pb1_sb = singles.tile([128, 1], dt.float32, tag="pb1")
            pb2_sb = singles.tile([64, 1], dt.float32, tag="pb2")
            pb3_sb = singles.tile([10, 1], dt.float32, tag="pb3")
            ones18 = singles.tile([1, 8], dt.bfloat16, tag="ones18")
            nc.vector.memset(ones18, 1.0)
            ones1s = singles.tile([1, 128], dt.bfloat16, tag="ones1s")
            nc.vector.memset(ones1s, 1.0)

            # conv2 input tiles, one per (chunk parity, half):
            # [97=(b,ci)+bias, 16x', 16s, 18y'']; b: 0=center(dx1), 1=L(dx0),
            # 2=R(dx2); lane 96 = ones (bias row).  Persistent pads: center x'
            # slots 0,15 zero; center y'' slots 1,16 zero (L/R copies propagate).
            in2 = [singles.tile([97, HALF, 18, 14], dt.bfloat16, tag=f"in2_{i}",
                                name=f"in2_{i}") for i in range(4)]
            for t in in2:
                nc.gpsimd.dma_start(out=t[0:32, :, 1:2, :], in_=zrow[:, :])
                nc.gpsimd.dma_start(out=t[0:32, :, 16:17, :], in_=zrow[:, :])
                nc.gpsimd.dma_start(
                    out=t[96:97].rearrange("p s y x -> p (s y x)"), in_=onerow[:, :])

            # fc stationary operand [128=(par,c), 128smp, 28i]
            p2full = singles.tile([128, B_CORE, 28], dt.bfloat16, tag="p2full")

            with tc.tile_pool(name="ximp", bufs=2) as ximpool, \
                 tc.tile_pool(name="p1xp", bufs=2) as p1xpool, \
                 tc.tile_pool(name="oddp", bufs=4) as oddpool, \
                 tc.tile_pool(name="scrp", bufs=2) as scrpool, \
                 tc.tile_pool(name="c2xp", bufs=2) as c2xpool, \
                 tc.tile_pool(name="c2yp", bufs=2) as c2ypool, \
                 tc.tile_pool(name="scr2p", bufs=2) as scr2pool, \
                 tc.tile_pool(name="ps1", bufs=4, space="PSUM") as psum1, \
                 tc.tile_pool(name="ps2", bufs=4, space="PSUM") as psum2:
                def emit_c1_slot(st, p):
                    xim_sb, p1x = st["xim"], st["p1x"]
                    for h in range(2):
                        q0, q1 = (0, 7) if h == 0 else (7, 14)
                        c1p = psum1.tile([128, 512], dt.float32, tag="c1p")
                        c1v = c1p[:, 0:392].rearrange(
                            "p (s q x) -> p s q x", s=2, q=7, x=28)
                        nc.tensor.matmul(
                            c1v[0:64], w16_sb,
                            xim_sb[:, 2 * p:2 * p + 2, q0:q1, :],
                            start=True, stop=True)
                        nc.tensor.matmul(
                            c1v[64:128], w16_sb,
                            xim_sb[:, HALF + 2 * p:HALF + 2 * p + 2, q0:q1, :],
                            start=True, stop=True, skip_group_check=True)
                        # drain + x-pool: ACT extracts odd-x, DVE stt + relu
                        cph = c1v.rearrange(
                            "p s q (xp two) -> p s q xp two", two=2)
                        oddt = oddpool.tile([128, 2, 7, 14], dt.bfloat16,
                                            tag="oddt")
                        nc.scalar.activation(oddt, cph[:, :, :, :, 1], AF.Copy)
                        nc.vector.scalar_tensor_tensor(
                            p1x[:, 2 * p:2 * p + 2, q0:q1, :],
                            cph[:, :, :, :, 0], 0.0, oddt,
                            ALU.max, ALU.max)

                def emit_fold(st):
                    p1x, tA, tB = st["p1x"], st["tA"], st["tB"]
                    scr = scrpool.tile([32, 3, HALF, 14, 14], dt.bfloat16,
                                       tag="scr")
                    nc.gpsimd.dma_start(out=scr[:, 0], in_=p1x[32:64])
                    nc.gpsimd.dma_start(out=scr[:, 1], in_=p1x[64:96])
                    nc.gpsimd.dma_start(out=scr[:, 2], in_=p1x[96:128])
                    for t2, in0, in1 in ((tA, p1x[0:32], scr[:, 0]),
                                         (tB, scr[:, 1], scr[:, 2])):
                        nc.vector.tensor_tensor(
                            t2[0:32, :, 2:16, :], in0, in1, ALU.max)
                        # L/R shifted blocks: flat one-element shift (fat runs),
                        # then re-zero the wrapped pad column
                        tf = t2.rearrange("p s y x -> p (s y x)")
                        nc.sync.dma_start(out=tf[32:64, 1:4032],
                                          in_=tf[0:32, 0:4031])
                        nc.sync.dma_start(out=tf[64:96, 0:4031],
                                          in_=tf[0:32, 1:4032])
                        nc.vector.memset(t2[32:64, :, :, 0:1], 0.0)
                        nc.vector.memset(t2[64:96, :, :, 13:14], 0.0)

                def emit_c2_slot(st, t):
                    tA, tB, c2x = st["tA"], st["tB"], st["c2x"]
                    c2p = psum2.tile([128, 392], dt.float32, tag="c2p")
                    c2v = c2p.rearrange("p (s y x) -> p s y x", s=2, y=14, x=14)
                    for dy in range(3):
                        rhsA = tA[:, 2 * t:2 * t + 2, dy + 1:dy + 15, :]
                        rhsB = tB[:, 2 * t:2 * t + 2, dy + 1:dy + 15, :]
                        nc.tensor.matmul(c2v[0:64], w2_sb[:, dy, :], rhsA,
                                         start=(dy == 0), stop=(dy == 2))
                        nc.tensor.matmul(c2v[64:128], w2_sb[:, dy, :], rhsB,
                                         start=(dy == 0), stop=(dy == 2),
                                         skip_group_check=True)
                    cp = c2v.rearrange("p s y (xp two) -> p s y xp two", two=2)
                    nc.vector.tensor_reduce(
                        c2x[:, 2 * t:2 * t + 2, 0:14, :], cp,
                        mybir.AxisListType.X, ALU.max)

                def emit_c2_post(st, ci):
                    # parity y-folds (reduce) + relu splits (ACT)
                    # even pixels Y=2Yq: raw rows (4Yq, 4Yq+1), Yq 0..3
                    # odd  pixels Y=2Yq+1: raw rows (4Yq+2, 4Yq+3), Yq 0..2
                    c2x = st["c2x"]
                    scr2 = scr2pool.tile([128, HALF, 28], dt.bfloat16, tag="scr2")
                    c2ye = c2ypool.tile([128, HALF, 4, 7], dt.bfloat16,
                                        tag="c2ye", name="c2ye")
                    c2yo = c2ypool.tile([128, HALF, 3, 7], dt.bfloat16,
                                        tag="c2yo", name="c2yo")
                    s0 = ci * CH
                    cyv = c2x.rearrange("p s (yq f) x -> p s yq x f", f=4)
                    nc.vector.tensor_reduce(c2ye, cyv[:, :, :, :, 0:2],
                                            mybir.AxisListType.X, ALU.max)
                    nc.vector.tensor_reduce(c2yo, cyv[:, :, 0:3, :, 2:4],
                                            mybir.AxisListType.X, ALU.max)
                    nc.scalar.activation(p2full[0:64, s0:s0 + HALF, :],
                                         c2ye[0:64], AF.Relu)
                    nc.scalar.activation(scr2[0:64, :, 0:21], c2yo[0:64],
                                         AF.Relu)
                    nc.scalar.activation(scr2[64:128, :, 0:28], c2ye[64:128],
                                         AF.Relu)
                    nc.scalar.activation(p2full[64:128, s0 + HALF:s0 + CH, 0:21],
                                         c2yo[64:128], AF.Relu)
                    nc.gpsimd.dma_start(out=p2full[64:128, s0:s0 + HALF, 0:21],
                                        in_=scr2[0:64, :, 0:21])
                    nc.gpsimd.dma_start(out=p2full[0:64, s0 + HALF:s0 + CH, :],
                                        in_=scr2[64:128, :, :])
                    if ci == 0:
                        nc.vector.memset(p2full[64:128, :, 21:28], 0.0)

                states = {}
                for ci in range(NCH + 1):
                    a = ci if ci < NCH else None
                    b = ci - 1 if ci >= 1 else None
                    if a is not None:
                        xim_sb = ximpool.tile([13, CH, 14, 28], dt.bfloat16,
                                              tag="xim_sb")
                        nc.sync.dma_start(out=xim_sb,
                                          in_=xim[:, a * CH:(a + 1) * CH])
                        states[a] = {
                            "xim": xim_sb,
                            "p1x": p1xpool.tile([128, HALF, 14, 14], dt.bfloat16,
                                                tag="p1x", name="p1x"),
                            "tA": in2[2 * (a % 2)],
                            "tB": in2[2 * (a % 2) + 1],
                        }
                        if a == 1:
                            nc.gpsimd.dma_start(out=fcw_sb, in_=fcw[:, :, :])
                            nc.gpsimd.dma_start(out=fcb_sb, in_=fcb[:, :])
                            nc.gpsimd.dma_start(out=id_sb, in_=id128[:, :])
                        elif a == 2:
                            nc.gpsimd.dma_start(out=mrt_sb, in_=mrt[:, :, :, :])
                            nc.gpsimd.dma_start(out=mit_sb, in_=mit[:, :, :, :])
                            nc.gpsimd.dma_start(out=zext_sb, in_=zext[:, :, :])
                            nc.gpsimd.dma_start(out=p1t_sb, in_=p1t[:, :])
                            nc.gpsimd.dma_start(out=p2t_sb, in_=p2t[:, :])
                            nc.gpsimd.dma_start(out=p3t_sb, in_=p3t[:, :])
                            nc.gpsimd.dma_start(out=pb1_sb, in_=pb1[:, :])
                            nc.gpsimd.dma_start(out=pb2_sb, in_=pb2[:, :])
                            nc.gpsimd.dma_start(out=pb3_sb, in_=pb3[:, :])
                    if b is not None:
                        states[b]["c2x"] = c2xpool.tile(
                            [128, HALF, 16, 7], dt.bfloat16, tag="c2x",
                            name="c2x")
                    for k in range(8):
                        if a is not None:
                            emit_c1_slot(states[a], k)
                        if b is not None:
                            emit_c2_slot(states[b], k)
                    if a is not None:
                        emit_fold(states[a])
                    if b is not None:
                        emit_c2_post(states[b], b)
                        del states[b]

            # ---------------- dense tail ----------------
            with tc.tile_pool(name="tail", bufs=1) as tail, \
                 tc.tile_pool(name="psumT", bufs=1, space="PSUM") as psumT:
                fcp = psumT.tile([128, 256], dt.float32, tag="fcp")
                for i in range(28):
                    nc.tensor.matmul(fcp, p2full[:, :, i], fcw_sb[:, i, :],
                                     start=(i == 0), stop=False)
                nc.tensor.matmul(fcp, ones1s, fcb_sb, start=False, stop=True)
                ftT = tail.tile([128, 256], dt.bfloat16, tag="ftT")
                nc.scalar.activation(ftT, fcp, AF.Tanh)

                feats = tail.tile([128, 2, 128], dt.bfloat16, tag="feats")
                ftp = psumT.tile([128, 2, 128], dt.bfloat16, tag="ftp")
                for mt in range(2):
                    nc.tensor.transpose(ftp[:, mt], ftT[:, mt * 128:(mt + 1) * 128],
                                        id_sb)
                    nc.scalar.activation(feats[:, mt], ftp[:, mt], AF.Copy)

                sq = psumT.tile([128, 4, 128], dt.float32, tag="sq")
                srp = sq[:, 0:2]
                sip = sq[:, 2:4]
                for mt in range(2):
                    for kb in range(2):
                        nc.tensor.matmul(srp[:, mt], mrt_sb[:, kb, mt, :], feats[:, kb],
                                         start=(kb == 0), stop=(kb == 1))
                    for kb in range(2):
                        nc.tensor.matmul(sip[:, mt], mit_sb[:, kb, mt, :], feats[:, kb],
                                         start=(kb == 0), stop=(kb == 1))

                probs = tail.tile([128, 2, 128], dt.bfloat16, tag="probs")
                t1 = tail.tile([128, 2, 128], dt.float32, tag="sq_r")
                nc.scalar.activation(t1, srp, AF.Square)
                t2s = tail.tile([128, 2, 128], dt.float32, tag="sq_i")
                nc.scalar.activation(t2s, sip, AF.Square)
                nc.vector.tensor_tensor(probs, t1, t2s, ALU.add)

                qt = psumT.tile([8, 2, 128], dt.float32, tag="qt")
                qp = qt[:, 0]
                tp = qt[0:1, 1]
                for kb in range(2):
                    nc.tensor.matmul(qp, zext_sb[:, kb, 0:8], probs[:, kb],
                                     start=(kb == 0), stop=(kb == 1))
                for kb in range(2):
                    nc.tensor.matmul(tp, zext_sb[:, kb, 8:9], probs[:, kb],
                                     start=(kb == 0), stop=(kb == 1))

                recip = tail.tile([1, 128], dt.float32, tag="recip")
                nc.vector.reciprocal(recip, tp)
                recip_bf = tail.tile([1, 128], dt.bfloat16, tag="recip_bf")
                nc.vector.tensor_copy(out=recip_bf, in_=recip)
                bc = psumT.tile([8, 128], dt.float32, tag="bc")
                nc.tensor.matmul(bc, ones18, recip_bf, start=True, stop=True)
                bc_sb = tail.tile([8, 128], dt.float32, tag="bc_sb")
                nc.scalar.activation(bc_sb, bc, AF.Copy)

                qn = tail.tile([8, 128], dt.bfloat16, tag="qn")
                nc.vector.tensor_tensor(qn, qp[0:8, :], bc_sb, ALU.mult)

                zp = psumT.tile([128, 3, 128], dt.float32, tag="zp")
                z1p = zp[:, 0]
                z2p = zp[0:64, 1]
                z3p = zp[0:10, 2]
                nc.tensor.matmul(z1p, p1t_sb, qn, start=True, stop=True)
                z1 = tail.tile([128, 128], dt.bfloat16, tag="z1")
                nc.scalar.activation(z1, z1p, AF.Relu, bias=pb1_sb[:, 0:1])

                nc.tensor.matmul(z2p, p2t_sb, z1, start=True, stop=True)
                z2 = tail.tile([64, 128], dt.bfloat16, tag="z2")
                nc.scalar.activation(z2, z2p, AF.Relu, bias=pb2_sb[:, 0:1])

                nc.tensor.matmul(z3p, p3t_sb, z2, start=True, stop=True)
                osb = tail.tile([10, 128], dt.float32, tag="osb")
                nc.vector.tensor_scalar_add(osb, z3p, pb3_sb[:, 0:1])
                nc.sync.dma_start(out=out[:, :], in_=osb)

    nc.finalize()
    return nc


def _get_nc():
    if "nc" not in _CACHE:
        _CACHE["nc"] = _build_bass()
    return _CACHE["nc"]


def kernel(**inputs) -> np.ndarray:
    from concourse.bass_utils import run_bass_kernel_spmd

    in_maps = _host_prep(inputs)
    nc = _get_nc()
    res = run_bass_kernel_spmd(nc, in_maps, core_ids=list(range(NCORES)),
                               trace=bool(_CACHE.get("trace")))
    _CACHE["last_result"] = res
    outs = [r["out"].T for r in res.results]  # each [128, 10]
    return np.ascontiguousarray(np.concatenate(outs, axis=0), dtype=np.float32)
